# revision 1
# baseline (speedup 1.0000x reference)
"""Multi-head linear attention (Performer/FAVOR+) Bass kernel for 8x TRN2 cores.

Sharding: 8 cores = 4 batches x 2 head-groups. Core c handles batch c//2 and
heads [4*(c%2), 4*(c%2)+4).

Math notes (exact rewrites of the reference, not approximations):
  - omega is sqrt(64) * orthogonal, so Omega @ Omega.T = 64*I. Hence
    0.5*||q||^2 = ||q @ Omega.T||^2 / 128: the squared-sum term is computed
    from xw itself and the plain q/k projections are never needed.
  - The per-row scale exp(-sq_t) on phi(q), the global 1/sqrt(128) scale and
    (approximately) the +EPS term all cancel in out = qkv[..,:64]/qkv[..,64],
    so the q-side feature map is just exp(+-xw) (minus sign folded into the
    wqpm weights).
  - The k-side scale rho_s = exp(-ksq_s) is folded into v1 = [v, 1]*rho so
    kp is also just exp(+-kxw) and no per-column bias is needed.

Layouts: inputs are pre-transposed to f-major fp16 on the host (sharding /
layout prep). All projections contract f=512 over 4 chunks of 128 partitions.
kp/v1/qp/kv are bf16; PSUM accumulation is fp32.

Schedule: phase-Q projection work (qxw matmuls + exp) is interleaved into the
KV s-loop to fill dependency stalls; the qkv/normalize tail runs last.
PSUM budget (8 banks): big(4: kv accums, then qkv tiles) + kxw(1) + vps(1)
+ qx(2).
"""

import sys

import numpy as np

for _p in ("/opt/trn_rl_repo", "/root/.axon_site/_ro/trn_rl_repo"):
    try:
        import concourse  # noqa: F401
        break
    except ImportError:
        if _p not in sys.path:
            sys.path.insert(0, _p)

B, T, D, H = 4, 4096, 512, 8
DK = DV = 64
HPC = 4            # heads per core
NCH = 4            # f chunks (512 / 128)
P = 128
ST = T // P        # 32 s-tiles
TC = 8             # t chunks
TCW = T // TC      # 512

_CACHE = {}


def _build_program(reps=1):
    import concourse.mybir as mybir
    import concourse.tile as tile
    from concourse import bacc
    from contextlib import ExitStack

    dt = mybir.dt
    AF = mybir.ActivationFunctionType

    nc = bacc.Bacc("TRN2", target_bir_lowering=False, debug=False)

    qt_d = nc.dram_tensor("qt", [D, T], dt.float16, kind="ExternalInput")
    kt_d = nc.dram_tensor("kt", [D, T], dt.float16, kind="ExternalInput")
    vt_d = nc.dram_tensor("vt", [D, T], dt.float16, kind="ExternalInput")
    wqpm_d = nc.dram_tensor("wqpm", [HPC, NCH, P, P], dt.float16, kind="ExternalInput")
    wko_d = nc.dram_tensor("wko", [NCH, P, HPC * DK], dt.float16, kind="ExternalInput")
    wv_d = nc.dram_tensor("wv", [NCH, P, HPC * DV], dt.float16, kind="ExternalInput")
    out_d = nc.dram_tensor("out", [HPC * T * DV], dt.float32, kind="ExternalOutput")

    with tile.TileContext(nc) as tc, ExitStack() as ctx:
        const = ctx.enter_context(tc.tile_pool(name="const", bufs=1))
        work = ctx.enter_context(tc.tile_pool(name="work", bufs=3))
        psum = ctx.enter_context(tc.tile_pool(name="psum", bufs=1, space="PSUM"))
        for _rep in range(reps):
            _emit_body(nc, tc, const, work, psum, mybir, dt, AF,
                       qt_d, kt_d, vt_d, wqpm_d, wko_d, wv_d, out_d)

    nc.compile()
    return nc


def _emit_body(nc, tc, const, work, psum, mybir, dt, AF,
               qt_d, kt_d, vt_d, wqpm_d, wko_d, wv_d, out_d):
    if True:

        # persistent SBUF residents
        qt = const.tile([P, NCH, T], dt.float16)
        kt = const.tile([P, NCH, T], dt.float16)
        vt = const.tile([P, NCH, T], dt.float16)
        wqpm = const.tile([P, HPC, NCH, P], dt.float16)
        wko = const.tile([P, NCH, HPC * DK], dt.float16)
        wv = const.tile([P, NCH, HPC * DV], dt.float16)
        kv_sb = const.tile([P, HPC, DV + 1], dt.bfloat16)

        # Load order matters: k/v weights and the first k/v columns unblock
        # the first loop iterations; wqpm/qt follow for the first q-chunk;
        # later 512-column segments stream in interleaved in the order the
        # loop consumes them.
        nc.sync.dma_start(out=wko[:], in_=wko_d.ap().rearrange("c p n -> p c n"))
        nc.sync.dma_start(out=wv[:], in_=wv_d.ap().rearrange("c p n -> p c n"))
        for c in range(NCH):
            nc.sync.dma_start(out=kt[:, c, 0:256], in_=kt_d[c * P:(c + 1) * P, 0:256])
        for c in range(NCH):
            nc.sync.dma_start(out=vt[:, c, 0:256], in_=vt_d[c * P:(c + 1) * P, 0:256])
        nc.sync.dma_start(out=wqpm[:], in_=wqpm_d.ap().rearrange("h c p m -> p h c m"))
        for c in range(NCH):
            nc.sync.dma_start(out=qt[:, c, 0:512], in_=qt_d[c * P:(c + 1) * P, 0:512])
        for c in range(NCH):
            nc.sync.dma_start(out=kt[:, c, 256:512], in_=kt_d[c * P:(c + 1) * P, 256:512])
        for c in range(NCH):
            nc.sync.dma_start(out=vt[:, c, 256:512], in_=vt_d[c * P:(c + 1) * P, 256:512])
        for psl in (slice(512, 1536), slice(1536, 2560), slice(2560, 3584),
                    slice(3584, T)):
            for c in range(NCH):
                nc.sync.dma_start(out=kt[:, c, psl], in_=kt_d[c * P:(c + 1) * P, psl])
            for c in range(NCH):
                nc.sync.dma_start(out=vt[:, c, psl], in_=vt_d[c * P:(c + 1) * P, psl])
            for c in range(NCH):
                nc.sync.dma_start(out=qt[:, c, psl], in_=qt_d[c * P:(c + 1) * P, psl])

        # Single-bank PSUM accumulator: kv[h] at columns [h*65, h*65+65).
        # first_mm clears has_written at BANK granularity, so interleaved
        # per-head groups must NOT use start=True: memset the bank once and
        # accumulate from the first matmul (overwrite-on-clear / add-on-set
        # per-element semantics make both stale-flag cases correct).
        kv_big = psum.tile([P, HPC, DV + 1], dt.float32, tag="kvacc", bufs=1)
        nc.vector.memset(kv_big[:], 0.0)

        qpt_tiles = []

        def emit_q_chunk(tcx):
            tsl = slice(tcx * TCW, (tcx + 1) * TCW)
            qpt = work.tile([P, HPC, TCW], dt.bfloat16, tag="qpt", bufs=TC,
                            name=f"qpt{tcx}")
            qpt_tiles.append(qpt)
            for h in range(HPC):
                qx = psum.tile([P, TCW], dt.float32, tag="qx", bufs=3,
                               name="qx")
                for c in range(NCH):
                    nc.tensor.matmul(
                        qx[:], wqpm[:, h, c, :], qt[:, c, tsl],
                        start=(c == 0), stop=(c == NCH - 1),
                    )
                nc.scalar.activation(qpt[:, h, :], qx[:], AF.Exp)

        # ---------------- phase KV (with q-projection work interleaved) -----
        # Two s-tiles are processed per "pair" iteration (halves the DVE/ACT
        # op count). Engines execute their queues IN ORDER, so cross-engine
        # dependencies are software-pipelined: v1 (needs rho from ACT) is
        # emitted one pair late on DVE, the kv matmuls (need v1) one pair
        # later still on PE. kp-exps are emitted before rho on ACT since
        # they only need kxw.
        NP_ = ST // 2    # 16 pairs
        stage = {}       # pi -> dict of tiles

        def emit_v1(pi):
            st_ = stage[pi]
            v1 = work.tile([P, 2, HPC, DV + 1], dt.bfloat16, tag="v1",
                           name="v1")
            nc.vector.tensor_mul(
                v1[:, :, :, 0:DV], st_["v_ps"],
                st_["rho"][:].broadcast_to([P, 2, HPC, DV])
            )
            nc.vector.tensor_copy(v1[:, :, :, DV:DV + 1], st_["rho"][:])
            st_["v1"] = v1

        def emit_kv(pi):
            st_ = stage.pop(pi)
            for p_ in range(2):
                si = 2 * pi + p_
                for h in range(HPC):
                    nc.tensor.matmul(
                        kv_big[:, h, :], st_["kp"][:, p_, h, :],
                        st_["v1"][:, p_, h, :],
                        start=False, stop=(si == ST - 1),
                        skip_group_check=True,
                    )

        for pi in range(NP_):
            # kxw and v share PSUM banks: [..., 0:64] = kxw, 64:128 = v
            kxwv = psum.tile([P, 2, HPC, 2 * DK], dt.float32, tag="kxwv",
                             bufs=2, name="kxwv")
            kxw = kxwv[:, :, :, 0:DK]
            v_ps = kxwv[:, :, :, DK:2 * DK]
            for p_ in range(2):
                ssl = slice((2 * pi + p_) * P, (2 * pi + p_ + 1) * P)
                for c in range(NCH):
                    nc.tensor.matmul(
                        kxwv[:, p_, :, 0:DK], kt[:, c, ssl], wko[:, c, :],
                        start=(c == 0), stop=(c == NCH - 1),
                    )
                for c in range(NCH):
                    nc.tensor.matmul(
                        kxwv[:, p_, :, DK:2 * DK], vt[:, c, ssl], wv[:, c, :],
                        start=(c == 0), stop=(c == NCH - 1),
                    )
            if pi >= 2:
                emit_kv(pi - 2)

            kp = work.tile([P, 2, HPC, 2 * DK], dt.bfloat16, tag="kp", bufs=3)
            nc.scalar.activation(kp[:, :, :, 0:DK], kxw, AF.Exp, scale=1.0)
            nc.scalar.activation(kp[:, :, :, DK:2 * DK], kxw, AF.Exp,
                                 scale=-1.0)

            sqsc = work.tile([P, 2, HPC, DK], dt.float32, tag="sqsc")
            nc.scalar.activation(sqsc[:], kxw, AF.Square)
            ksqr = work.tile([P, 2, HPC, 1], dt.float32, tag="ksqr")
            nc.vector.reduce_sum(ksqr[:], sqsc[:], axis=mybir.AxisListType.X)
            rho = work.tile([P, 2, HPC, 1], dt.float32, tag="rho")
            nc.scalar.activation(rho[:], ksqr[:], AF.Exp, scale=-1.0 / 128.0)

            stage[pi] = {"v_ps": v_ps, "rho": rho, "kp": kp}
            if pi >= 1:
                emit_v1(pi - 1)

            if pi % 2 == 1:
                emit_q_chunk(pi // 2)

        emit_v1(NP_ - 1)
        emit_kv(NP_ - 2)
        emit_kv(NP_ - 1)
        nc.vector.tensor_copy(kv_sb[:], kv_big[:])

        # ---------------- tail: qkv + normalize + store ---------------------
        qkv_tags = ["qx", "qx", "qx", "kxwv", "kxwv"]
        qkv_bufs = {"qx": 3, "kxwv": 2}
        qkv_i = 0
        for tcx in range(TC):
            qpt = qpt_tiles[tcx]
            oacc = work.tile([P, HPC, 4, DV], dt.float32, tag="oacc", bufs=4)
            for tt in range(4):
                tg = qkv_tags[qkv_i % 5]
                qkv_i += 1
                qkv = psum.tile([P, HPC, P], dt.float32, tag=tg,
                                bufs=qkv_bufs[tg], name="qkv")
                ttsl = slice(tt * P, (tt + 1) * P)
                for h in range(HPC):
                    nc.tensor.matmul(
                        qkv[:, h, 0:DV + 1],
                        qpt[:, h, ttsl],
                        kv_sb[:, h, :],
                    )
                recip = work.tile([P, HPC, 1], dt.float32, tag="recip")
                nc.vector.reciprocal(recip[:], qkv[:, :, DV:DV + 1])
                if tt % 2 == 0:
                    nc.vector.tensor_mul(
                        oacc[:, :, tt, :], qkv[:, :, 0:DV],
                        recip[:].broadcast_to([P, HPC, DV]),
                    )
                else:
                    for h in range(HPC):
                        nc.scalar.mul(oacc[:, h, tt, :], qkv[:, h, 0:DV],
                                      recip[:, h, :])
            for h in range(HPC):
                ofs = h * T * DV + tcx * TCW * DV
                nc.sync.dma_start(
                    out=out_d.ap()[ofs:ofs + TCW * DV].rearrange(
                        "(tt p d) -> p tt d", tt=4, p=P
                    ),
                    in_=oacc[:, h, :, :],
                )


def _get_program(reps=1):
    if reps not in _CACHE:
        _CACHE[reps] = _build_program(reps)
    return _CACHE[reps]


def _prep_core_inputs(query, value, key, wqo, wko, wv_w, core):
    b, hg = core // 2, core % 2
    hs = slice(hg * HPC, (hg + 1) * HPC)

    qT = np.ascontiguousarray(query[b].T.astype(np.float16))   # (512, 4096)
    kT = np.ascontiguousarray(key[b].T.astype(np.float16))
    vT = np.ascontiguousarray(value[b].T.astype(np.float16))

    wqo_c = wqo[hs]                                            # (4, 512, 64)
    wqpm = np.concatenate([wqo_c, -wqo_c], axis=2)             # (4, 512, 128)
    wqpm = np.ascontiguousarray(
        wqpm.reshape(HPC, NCH, P, P).astype(np.float16))       # (h, c, p, m)

    wko_c = np.concatenate(list(wko[hs]), axis=1)              # (512, 256)
    wko_c = np.ascontiguousarray(
        wko_c.reshape(NCH, P, HPC * DK).astype(np.float16))
    wv_c = np.concatenate(list(wv_w[hs]), axis=1)              # (512, 256)
    wv_c = np.ascontiguousarray(
        wv_c.reshape(NCH, P, HPC * DV).astype(np.float16))

    return {"qt": qT, "kt": kT, "vt": vT,
            "wqpm": wqpm, "wko": wko_c, "wv": wv_c}


def kernel(query, value, key, wq, wv, wk, omega):
    from concourse.bass_utils import run_bass_kernel_spmd

    query = np.asarray(query, np.float32)
    value = np.asarray(value, np.float32)
    key = np.asarray(key, np.float32)
    wq = np.asarray(wq, np.float32)
    wv = np.asarray(wv, np.float32)
    wk = np.asarray(wk, np.float32)
    omega = np.asarray(omega, np.float32)

    nc = _get_program()

    wqo = np.einsum("hfk,mk->hfm", wq, omega)                  # (8, 512, 64)
    wko = np.einsum("hfk,mk->hfm", wk, omega)

    in_maps = [
        _prep_core_inputs(query, value, key, wqo, wko, wv, core)
        for core in range(8)
    ]
    res = run_bass_kernel_spmd(nc, in_maps, core_ids=list(range(8)))

    out = np.empty((B, T, D), np.float32)
    for core in range(8):
        b, hg = core // 2, core % 2
        out[b, hg * 2048:(hg + 1) * 2048, :] = (
            res.results[core]["out"].reshape(2048, 512))
    return out



# revision 48
# speedup vs baseline: 971.5473x; 971.5473x over previous
"""Multi-head linear attention (Performer/FAVOR+) Bass kernel for 8x TRN2 cores.

Sharding: 8 cores = 4 batches x 2 head-groups. Core c handles batch c//2 and
heads [4*(c%2), 4*(c%2)+4).

Math notes (exact rewrites of the reference, not approximations):
  - omega is sqrt(64) * orthogonal, so Omega @ Omega.T = 64*I. Hence
    0.5*||q||^2 = ||q @ Omega.T||^2 / 128: the squared-sum term is computed
    from xw itself and the plain q/k projections are never needed.
  - The per-row scale exp(-sq_t) on phi(q), the global 1/sqrt(128) scale and
    (approximately) the +EPS term all cancel in out = qkv[..,:64]/qkv[..,64],
    so the q-side feature map is just exp(+-xw) (minus sign folded into the
    wqpm weights).
  - The k-side scale rho_s = exp(-ksq_s) is folded into v1 = [v, 1]*rho so
    kp is also just exp(+-kxw) and no per-column bias is needed.

Layouts: inputs are pre-transposed to f-major fp16 on the host (sharding /
layout prep). All projections contract f=512 over 4 chunks of 128 partitions.
kp/v1/qp/kv are bf16; PSUM accumulation is fp32.

Schedule: phase-Q projection work (qxw matmuls + exp) is interleaved into the
KV s-loop to fill dependency stalls; the qkv/normalize tail runs last.
PSUM budget (8 banks): big(4: kv accums, then qkv tiles) + kxw(1) + vps(1)
+ qx(2).
"""

import sys

import numpy as np

for _p in ("/opt/trn_rl_repo", "/root/.axon_site/_ro/trn_rl_repo"):
    try:
        import concourse  # noqa: F401
        break
    except ImportError:
        if _p not in sys.path:
            sys.path.insert(0, _p)

B, T, D, H = 4, 4096, 512, 8
DK = DV = 64
HPC = 4            # heads per core
NCH = 4            # f chunks (512 / 128)
P = 128
ST = T // P        # 32 s-tiles
TC = 8             # t chunks
TCW = T // TC      # 512

_CACHE = {}


def _build_program(reps=1):
    import concourse.mybir as mybir
    import concourse.tile as tile
    from concourse import bacc
    from contextlib import ExitStack

    dt = mybir.dt
    AF = mybir.ActivationFunctionType

    nc = bacc.Bacc("TRN2", target_bir_lowering=False, debug=False)

    qt_d = nc.dram_tensor("qt", [D, T], dt.float16, kind="ExternalInput")
    kt_d = nc.dram_tensor("kt", [D, T], dt.float16, kind="ExternalInput")
    vt_d = nc.dram_tensor("vt", [D, T], dt.float16, kind="ExternalInput")
    wqp_d = nc.dram_tensor("wqp", [NCH, P, HPC * DK], dt.float16, kind="ExternalInput")
    wko_d = nc.dram_tensor("wko", [NCH, P, HPC * DK], dt.float16, kind="ExternalInput")
    wv_d = nc.dram_tensor("wv", [NCH, P, HPC * DV], dt.float16, kind="ExternalInput")
    out_d = nc.dram_tensor("out", [HPC * T * DV], dt.float32, kind="ExternalOutput")
    kvs_d = nc.dram_tensor("kvs", [P, HPC, DV + 1], dt.bfloat16, kind="Internal")

    with tile.TileContext(nc) as tc, ExitStack() as ctx:
        const = ctx.enter_context(tc.tile_pool(name="const", bufs=1))
        work = ctx.enter_context(tc.tile_pool(name="work", bufs=3))
        psum = ctx.enter_context(tc.tile_pool(name="psum", bufs=1, space="PSUM"))
        for _rep in range(reps):
            _emit_body(nc, tc, const, work, psum, mybir, dt, AF,
                       qt_d, kt_d, vt_d, wqp_d, wko_d, wv_d, out_d, kvs_d)

    nc.compile()
    return nc


def _emit_body(nc, tc, const, work, psum, mybir, dt, AF,
               qt_d, kt_d, vt_d, wqp_d, wko_d, wv_d, out_d, kvs_d):
    if True:

        # persistent SBUF residents
        qt = const.tile([P, NCH, T], dt.float16)
        kt = const.tile([P, NCH, T], dt.float16)
        vt = const.tile([P, NCH, T], dt.float16)
        wqp = const.tile([P, NCH, HPC * DK], dt.float16)
        wko = const.tile([P, NCH, HPC * DK], dt.float16)
        wv = const.tile([P, NCH, HPC * DV], dt.float16)
        kv_sb = const.tile([P, HPC, DV + 1], dt.bfloat16)
        # Block-diagonal kv for the packed qkv matmuls: for pair pr and
        # sign s, a [128, 130] rhs whose top-left 64x65 block is kv of head
        # 2pr and bottom-right 64x65 block is kv of head 2pr+1 (zeros
        # elsewhere). One matmul against the packed qpt tile then yields
        # both heads' qkv side by side, with all operands at partition 0.
        kvz = const.tile([P, 2, 2, 2 * (DV + 1)], dt.bfloat16)

        # Load order matters: k/v weights and the first k/v columns unblock
        # the first loop iterations; wqpm/qt follow for the first q-chunk;
        # later 512-column segments stream in interleaved in the order the
        # loop consumes them.
        qt_r = qt_d.ap().rearrange("(c p) t -> p c t", p=P)
        kt_r = kt_d.ap().rearrange("(c p) t -> p c t", p=P)
        vt_r = vt_d.ap().rearrange("(c p) t -> p c t", p=P)

        nc.sync.dma_start(out=wko[:], in_=wko_d.ap().rearrange("c p n -> p c n"))
        nc.sync.dma_start(out=wv[:], in_=wv_d.ap().rearrange("c p n -> p c n"))
        nc.sync.dma_start(out=kt[:, :, 0:256], in_=kt_r[:, :, 0:256])
        nc.sync.dma_start(out=vt[:, :, 0:256], in_=vt_r[:, :, 0:256])
        nc.sync.dma_start(out=wqp[:], in_=wqp_d.ap().rearrange("c p n -> p c n"))
        nc.sync.dma_start(out=qt[:, :, 0:512], in_=qt_r[:, :, 0:512])
        nc.sync.dma_start(out=kt[:, :, 256:512], in_=kt_r[:, :, 256:512])
        nc.sync.dma_start(out=vt[:, :, 256:512], in_=vt_r[:, :, 256:512])
        for psl in (slice(512, 1536), slice(1536, 2560), slice(2560, 3584),
                    slice(3584, T)):
            nc.sync.dma_start(out=kt[:, :, psl], in_=kt_r[:, :, psl])
            nc.sync.dma_start(out=vt[:, :, psl], in_=vt_r[:, :, psl])
            nc.sync.dma_start(out=qt[:, :, psl], in_=qt_r[:, :, psl])

        # Single-bank PSUM accumulator: kv[h] at columns [h*65, h*65+65).
        # first_mm clears has_written at BANK granularity, so interleaved
        # per-head groups must NOT use start=True: memset the bank once and
        # accumulate from the first matmul (overwrite-on-clear / add-on-set
        # per-element semantics make both stale-flag cases correct).
        kv_big = psum.tile([P, HPC, DV + 1], dt.float32, tag="kvacc", bufs=1)
        nc.vector.memset(kv_big[:], 0.0)

        qpt_tiles = []

        def emit_q_chunk(tcx):
            tsl = slice(tcx * TCW, (tcx + 1) * TCW)
            # [pair, sign, t]: partitions hold (head 2p: m 0..63,
            # head 2p+1: m 0..63); sign 0 = exp(+xw), 1 = exp(-xw).
            qpt = work.tile([P, 2, 2, TCW], dt.bfloat16, tag="qpt", bufs=TC,
                            name=f"qpt{tcx}")
            qpt_tiles.append(qpt)
            for pr in range(2):
                qx = psum.tile([P, TCW], dt.float32, tag="qx", bufs=3,
                               name="qx")
                for c in range(NCH):
                    nc.tensor.matmul(
                        qx[:], wqp[:, c, pr * P:(pr + 1) * P], qt[:, c, tsl],
                        start=(c == 0), stop=(c == NCH - 1),
                    )
                nc.scalar.activation(qpt[:, pr, 0, :], qx[:], AF.Exp)
                nc.scalar.activation(qpt[:, pr, 1, :], qx[:], AF.Exp,
                                     scale=-1.0)

        # ---------------- phase KV (with q-projection work interleaved) -----
        # Two s-tiles are processed per "pair" iteration (halves the DVE/ACT
        # op count). Engines execute their queues IN ORDER, so cross-engine
        # dependencies are software-pipelined: v1 (needs rho from ACT) is
        # emitted one pair late on DVE, the kv matmuls (need v1) one pair
        # later still on PE. kp-exps are emitted before rho on ACT since
        # they only need kxw.
        NP_ = ST // 2    # 16 pairs
        stage = {}       # pi -> dict of tiles

        def emit_v1(pi):
            st_ = stage[pi]
            v1 = work.tile([P, 2, HPC, DV + 1], dt.bfloat16, tag="v1",
                           name="v1")
            nc.vector.tensor_mul(
                v1[:, :, :, 0:DV], st_["v_ps"],
                st_["rho"][:].broadcast_to([P, 2, HPC, DV])
            )
            nc.vector.tensor_copy(v1[:, :, :, DV:DV + 1], st_["rho"][:])
            st_["v1"] = v1

        def emit_kv(pi):
            st_ = stage.pop(pi)
            for p_ in range(2):
                si = 2 * pi + p_
                for h in range(HPC):
                    # lhsT = [s, 128]: head h's features in [+64, -64] order
                    # (contiguous, matches kv row order).
                    nc.tensor.matmul(
                        kv_big[:, h, :], st_["kp"][:, p_, h, :, :],
                        st_["v1"][:, p_, h, :],
                        start=False, stop=(si == ST - 1),
                        skip_group_check=True,
                    )

        for pi in range(NP_):
            # kxw and v as contiguous 256-wide slabs per s-tile:
            # [:, p_, 0, :] = kxw (4h x 64), [:, p_, 1, :] = v (4h x 64).
            kxwv = psum.tile([P, 2, 2, HPC * DK], dt.float32, tag="kxwv",
                             bufs=2, name="kxwv")
            kxw = kxwv[:, :, 0, :]
            kxw4 = kxw.rearrange("p a (h m) -> p a h m", h=HPC)
            v_ps = kxwv[:, :, 1, :].rearrange("p a (h m) -> p a h m", h=HPC)
            for p_ in range(2):
                ssl = slice((2 * pi + p_) * P, (2 * pi + p_ + 1) * P)
                for c in range(NCH):
                    nc.tensor.matmul(
                        kxwv[:, p_, 0, :], kt[:, c, ssl], wko[:, c, :],
                        start=(c == 0), stop=(c == NCH - 1),
                    )
                for c in range(NCH):
                    nc.tensor.matmul(
                        kxwv[:, p_, 1, :], vt[:, c, ssl], wv[:, c, :],
                        start=(c == 0), stop=(c == NCH - 1),
                    )
            if pi >= 2:
                emit_kv(pi - 2)

            # kp layout [p_, h, sign, m]: head h's 128 features contiguous
            # in [+64, -64] order so the kv matmul stationary slice is 2D.
            kp = work.tile([P, 2, HPC, 2, DK], dt.bfloat16, tag="kp", bufs=3)
            nc.scalar.activation(kp[:, :, :, 0, :], kxw4, AF.Exp, scale=1.0)
            nc.scalar.activation(kp[:, :, :, 1, :], kxw4, AF.Exp, scale=-1.0)

            sqsc = work.tile([P, 2, HPC, DK], dt.float32, tag="sqsc")
            nc.scalar.activation(sqsc[:], kxw4, AF.Square)
            ksqr = work.tile([P, 2, HPC, 1], dt.float32, tag="ksqr")
            nc.vector.reduce_sum(ksqr[:], sqsc[:], axis=mybir.AxisListType.X)
            rho = work.tile([P, 2, HPC, 1], dt.float32, tag="rho")
            nc.scalar.activation(rho[:], ksqr[:], AF.Exp, scale=-1.0 / 128.0)

            stage[pi] = {"v_ps": v_ps, "rho": rho, "kp": kp}
            if pi >= 1:
                emit_v1(pi - 1)

            if pi % 2 == 1:
                emit_q_chunk(pi // 2)

        emit_v1(NP_ - 1)
        emit_kv(NP_ - 2)
        emit_kv(NP_ - 1)
        nc.vector.tensor_copy(kv_sb[:], kv_big[:])
        # Build kvz via a DRAM round-trip (cross-partition moves need DMA;
        # matmul operands must then sit at partition base 0).
        nc.vector.memset(kvz[:], 0.0)
        nc.sync.dma_start(out=kvs_d.ap(), in_=kv_sb[:])
        for sg in range(2):
            for hf in range(2):
                dsl = slice(hf * (DV + 1), (hf + 1) * (DV + 1))
                nc.sync.dma_start(
                    out=kvz[hf * DK:(hf + 1) * DK, sg, :, dsl],
                    in_=kvs_d.ap().rearrange(
                        "p (pr hf) d -> p hf pr d", hf=2)[
                        sg * DK:(sg + 1) * DK, hf, :, :])

        # ---------------- tail: qkv + normalize + store ---------------------
        # Per-t-tile qkv through 5 rotating 1-bank PSUM tiles; normalize
        # alternates DVE / ACT; output DMAs spread across SP + ACT queues.
        out_r = out_d.ap().rearrange(
            "(h tc tt p d) -> h tc p tt d", h=HPC, tc=TC, tt=4, p=P, d=DV)
        qkv_tags = ["qx", "qx", "qx", "kxwv", "kxwv"]
        qkv_bufs = {"qx": 3, "kxwv": 2}
        qkv_i = 0
        for tcx in range(TC):
            qpt = qpt_tiles[tcx]
            oacc = work.tile([P, HPC, 4, DV], dt.float32, tag="oacc", bufs=4)
            oacc4 = oacc[:].rearrange("p (pr hf) tt d -> p pr hf tt d", hf=2)
            for tt in range(4):
                tg = qkv_tags[qkv_i % 5]
                qkv_i += 1
                # [pr, hd, 65]: one matmul per (pair, sign) yields both
                # heads of the pair via the block-diagonal kvz rhs.
                qkv = psum.tile([P, 2, 2, DV + 1], dt.float32, tag=tg,
                                bufs=qkv_bufs[tg], name="qkv")
                ttsl = slice(tt * P, (tt + 1) * P)
                for pr in range(2):
                    for sg in range(2):
                        nc.tensor.matmul(
                            qkv[:, pr, :, :],
                            qpt[:, pr, sg, ttsl],
                            kvz[:, sg, pr, :],
                            start=(sg == 0), stop=(sg == 1),
                        )
                recip = work.tile([P, 2, 2, 1], dt.float32, tag="recip")
                nc.vector.reciprocal(recip[:], qkv[:, :, :, DV:DV + 1])
                nc.vector.tensor_mul(
                    oacc4[:, :, :, tt, :], qkv[:, :, :, 0:DV],
                    recip[:].broadcast_to([P, 2, 2, DV]),
                )
            out_qs = [nc.sync, nc.scalar, nc.scalar, nc.sync]
            for h in range(HPC):
                out_qs[h].dma_start(out=out_r[h, tcx], in_=oacc[:, h, :, :])

def _get_program(reps=1):
    if reps not in _CACHE:
        _CACHE[reps] = _build_program(reps)
    return _CACHE[reps]


def _prep_core_inputs(query, value, key, wqo, wko, wv_w, core):
    b, hg = core // 2, core % 2
    hs = slice(hg * HPC, (hg + 1) * HPC)

    qT = np.ascontiguousarray(query[b].T.astype(np.float16))   # (512, 4096)
    kT = np.ascontiguousarray(key[b].T.astype(np.float16))
    vT = np.ascontiguousarray(value[b].T.astype(np.float16))

    wqp_c = np.concatenate(list(wqo[hs]), axis=1)              # (512, 256)
    wqp_c = np.ascontiguousarray(
        wqp_c.reshape(NCH, P, HPC * DK).astype(np.float16))
    wko_c = np.concatenate(list(wko[hs]), axis=1)              # (512, 256)
    wko_c = np.ascontiguousarray(
        wko_c.reshape(NCH, P, HPC * DK).astype(np.float16))
    wv_c = np.concatenate(list(wv_w[hs]), axis=1)              # (512, 256)
    wv_c = np.ascontiguousarray(
        wv_c.reshape(NCH, P, HPC * DV).astype(np.float16))

    return {"qt": qT, "kt": kT, "vt": vT,
            "wqp": wqp_c, "wko": wko_c, "wv": wv_c}


def kernel(query, value, key, wq, wv, wk, omega):
    from concourse.bass_utils import run_bass_kernel_spmd

    query = np.asarray(query, np.float32)
    value = np.asarray(value, np.float32)
    key = np.asarray(key, np.float32)
    wq = np.asarray(wq, np.float32)
    wv = np.asarray(wv, np.float32)
    wk = np.asarray(wk, np.float32)
    omega = np.asarray(omega, np.float32)

    nc = _get_program()

    wqo = np.einsum("hfk,mk->hfm", wq, omega)                  # (8, 512, 64)
    wko = np.einsum("hfk,mk->hfm", wk, omega)

    in_maps = [
        _prep_core_inputs(query, value, key, wqo, wko, wv, core)
        for core in range(8)
    ]
    res = run_bass_kernel_spmd(nc, in_maps, core_ids=list(range(8)))

    out = np.empty((B, T, D), np.float32)
    for core in range(8):
        b, hg = core // 2, core % 2
        out[b, hg * 2048:(hg + 1) * 2048, :] = (
            res.results[core]["out"].reshape(2048, 512))
    return out


# revision 49
# speedup vs baseline: 1014.6078x; 1.0443x over previous
"""Multi-head linear attention (Performer/FAVOR+) Bass kernel for 8x TRN2 cores.

Sharding: 8 cores = 4 batches x 2 head-groups. Core c handles batch c//2 and
heads [4*(c%2), 4*(c%2)+4).

Math notes (exact rewrites of the reference, not approximations):
  - omega is sqrt(64) * orthogonal, so Omega @ Omega.T = 64*I. Hence
    0.5*||q||^2 = ||q @ Omega.T||^2 / 128: the squared-sum term is computed
    from xw itself and the plain q/k projections are never needed.
  - The per-row scale exp(-sq_t) on phi(q), the global 1/sqrt(128) scale and
    (approximately) the +EPS term all cancel in out = qkv[..,:64]/qkv[..,64],
    so the q-side feature map is just exp(+-xw) (minus sign folded into the
    wqpm weights).
  - The k-side scale rho_s = exp(-ksq_s) is folded into v1 = [v, 1]*rho so
    kp is also just exp(+-kxw) and no per-column bias is needed.

Layouts: inputs are pre-transposed to f-major fp16 on the host (sharding /
layout prep). All projections contract f=512 over 4 chunks of 128 partitions.
kp/v1/qp/kv are bf16; PSUM accumulation is fp32.

Schedule: phase-Q projection work (qxw matmuls + exp) is interleaved into the
KV s-loop to fill dependency stalls; the qkv/normalize tail runs last.
PSUM budget (8 banks): big(4: kv accums, then qkv tiles) + kxw(1) + vps(1)
+ qx(2).
"""

import sys

import numpy as np

for _p in ("/opt/trn_rl_repo", "/root/.axon_site/_ro/trn_rl_repo"):
    try:
        import concourse  # noqa: F401
        break
    except ImportError:
        if _p not in sys.path:
            sys.path.insert(0, _p)

B, T, D, H = 4, 4096, 512, 8
DK = DV = 64
HPC = 4            # heads per core
NCH = 4            # f chunks (512 / 128)
P = 128
ST = T // P        # 32 s-tiles
TC = 8             # t chunks
TCW = T // TC      # 512

_CACHE = {}


def _build_program(reps=1):
    import concourse.mybir as mybir
    import concourse.tile as tile
    from concourse import bacc
    from contextlib import ExitStack

    dt = mybir.dt
    AF = mybir.ActivationFunctionType

    nc = bacc.Bacc("TRN2", target_bir_lowering=False, debug=False)

    qt_d = nc.dram_tensor("qt", [D, T], dt.float16, kind="ExternalInput")
    kt_d = nc.dram_tensor("kt", [D, T], dt.float16, kind="ExternalInput")
    vt_d = nc.dram_tensor("vt", [D, T], dt.float16, kind="ExternalInput")
    wqp_d = nc.dram_tensor("wqp", [NCH, P, HPC * DK], dt.float16, kind="ExternalInput")
    wko_d = nc.dram_tensor("wko", [NCH, P, HPC * DK], dt.float16, kind="ExternalInput")
    wv_d = nc.dram_tensor("wv", [NCH, P, HPC * DV], dt.float16, kind="ExternalInput")
    out_d = nc.dram_tensor("out", [HPC * T * DV], dt.float32, kind="ExternalOutput")
    kvs_d = nc.dram_tensor("kvs", [P, HPC, DV + 1], dt.bfloat16, kind="Internal")

    with tile.TileContext(nc) as tc, ExitStack() as ctx:
        const = ctx.enter_context(tc.tile_pool(name="const", bufs=1))
        work = ctx.enter_context(tc.tile_pool(name="work", bufs=3))
        psum = ctx.enter_context(tc.tile_pool(name="psum", bufs=1, space="PSUM"))
        for _rep in range(reps):
            _emit_body(nc, tc, const, work, psum, mybir, dt, AF,
                       qt_d, kt_d, vt_d, wqp_d, wko_d, wv_d, out_d, kvs_d)

    nc.compile()
    return nc


def _emit_body(nc, tc, const, work, psum, mybir, dt, AF,
               qt_d, kt_d, vt_d, wqp_d, wko_d, wv_d, out_d, kvs_d):
    if True:

        # persistent SBUF residents
        qt = const.tile([P, NCH, T], dt.float16)
        kt = const.tile([P, NCH, T], dt.float16)
        vt = const.tile([P, NCH, T], dt.float16)
        wqp = const.tile([P, NCH, HPC * DK], dt.float16)
        wko = const.tile([P, NCH, HPC * DK], dt.float16)
        wv = const.tile([P, NCH, HPC * DV], dt.float16)
        kv_sb = const.tile([P, HPC, DV + 1], dt.bfloat16)
        # Block-diagonal kv for the packed qkv matmuls: for pair pr and
        # sign s, a [128, 130] rhs whose top-left 64x65 block is kv of head
        # 2pr and bottom-right 64x65 block is kv of head 2pr+1 (zeros
        # elsewhere). One matmul against the packed qpt tile then yields
        # both heads' qkv side by side, with all operands at partition 0.
        kvz = const.tile([P, 2, 2, 2 * (DV + 1)], dt.bfloat16)

        # Load order matters: k/v weights and the first k/v columns unblock
        # the first loop iterations; wqpm/qt follow for the first q-chunk;
        # later 512-column segments stream in interleaved in the order the
        # loop consumes them.
        qt_r = qt_d.ap().rearrange("(c p) t -> p c t", p=P)
        kt_r = kt_d.ap().rearrange("(c p) t -> p c t", p=P)
        vt_r = vt_d.ap().rearrange("(c p) t -> p c t", p=P)

        nc.sync.dma_start(out=wko[:], in_=wko_d.ap().rearrange("c p n -> p c n"))
        nc.sync.dma_start(out=wv[:], in_=wv_d.ap().rearrange("c p n -> p c n"))
        nc.sync.dma_start(out=kt[:, :, 0:256], in_=kt_r[:, :, 0:256])
        nc.sync.dma_start(out=vt[:, :, 0:256], in_=vt_r[:, :, 0:256])
        nc.sync.dma_start(out=wqp[:], in_=wqp_d.ap().rearrange("c p n -> p c n"))
        nc.sync.dma_start(out=qt[:, :, 0:512], in_=qt_r[:, :, 0:512])
        nc.sync.dma_start(out=kt[:, :, 256:512], in_=kt_r[:, :, 256:512])
        nc.sync.dma_start(out=vt[:, :, 256:512], in_=vt_r[:, :, 256:512])
        for psl in (slice(512, 1536), slice(1536, 2560), slice(2560, 3584),
                    slice(3584, T)):
            nc.sync.dma_start(out=kt[:, :, psl], in_=kt_r[:, :, psl])
            nc.sync.dma_start(out=vt[:, :, psl], in_=vt_r[:, :, psl])
            nc.sync.dma_start(out=qt[:, :, psl], in_=qt_r[:, :, psl])

        # Single-bank PSUM accumulator: kv[h] at columns [h*65, h*65+65).
        # first_mm clears has_written at BANK granularity, so interleaved
        # per-head groups must NOT use start=True: memset the bank once and
        # accumulate from the first matmul (overwrite-on-clear / add-on-set
        # per-element semantics make both stale-flag cases correct).
        kv_big = psum.tile([P, HPC, DV + 1], dt.float32, tag="kvacc", bufs=1)
        nc.vector.memset(kv_big[:], 0.0)

        qpt_tiles = []

        def emit_q_chunk(tcx):
            tsl = slice(tcx * TCW, (tcx + 1) * TCW)
            # [pair, sign, t]: partitions hold (head 2p: m 0..63,
            # head 2p+1: m 0..63); sign 0 = exp(+xw), 1 = exp(-xw).
            qpt = work.tile([P, 2, 2, TCW], dt.bfloat16, tag="qpt", bufs=TC,
                            name=f"qpt{tcx}")
            qpt_tiles.append(qpt)
            for pr in range(2):
                qx = psum.tile([P, TCW], dt.float32, tag="qx", bufs=3,
                               name="qx")
                for c in range(NCH):
                    nc.tensor.matmul(
                        qx[:], wqp[:, c, pr * P:(pr + 1) * P], qt[:, c, tsl],
                        start=(c == 0), stop=(c == NCH - 1),
                    )
                nc.scalar.activation(qpt[:, pr, 0, :], qx[:], AF.Exp)
                nc.scalar.activation(qpt[:, pr, 1, :], qx[:], AF.Exp,
                                     scale=-1.0)

        # ---------------- phase KV (with q-projection work interleaved) -----
        # Two s-tiles are processed per "pair" iteration (halves the DVE/ACT
        # op count). Engines execute their queues IN ORDER, so cross-engine
        # dependencies are software-pipelined: v1 (needs rho from ACT) is
        # emitted one pair late on DVE, the kv matmuls (need v1) one pair
        # later still on PE. kp-exps are emitted before rho on ACT since
        # they only need kxw.
        NP_ = ST // 2    # 16 pairs
        stage = {}       # pi -> dict of tiles

        def emit_v1(pi):
            st_ = stage[pi]
            v1 = work.tile([P, 2, HPC, DV + 1], dt.bfloat16, tag="v1",
                           name="v1")
            nc.vector.tensor_mul(
                v1[:, :, :, 0:DV], st_["v_ps"],
                st_["rho"][:].broadcast_to([P, 2, HPC, DV])
            )
            nc.vector.tensor_copy(v1[:, :, :, DV:DV + 1], st_["rho"][:])
            st_["v1"] = v1

        def emit_kv(pi):
            st_ = stage.pop(pi)
            for p_ in range(2):
                si = 2 * pi + p_
                for h in range(HPC):
                    # lhsT = [s, 128]: head h's features in [+64, -64] order
                    # (contiguous, matches kv row order).
                    nc.tensor.matmul(
                        kv_big[:, h, :], st_["kp"][:, p_, h, :, :],
                        st_["v1"][:, p_, h, :],
                        start=False, stop=(si == ST - 1),
                        skip_group_check=True,
                    )

        for pi in range(NP_):
            # kxw and v as contiguous 256-wide slabs per s-tile:
            # [:, p_, 0, :] = kxw (4h x 64), [:, p_, 1, :] = v (4h x 64).
            kxwv = psum.tile([P, 2, 2, HPC * DK], dt.float32, tag="kxwv",
                             bufs=2, name="kxwv")
            kxw = kxwv[:, :, 0, :]
            kxw4 = kxw.rearrange("p a (h m) -> p a h m", h=HPC)
            v_ps = kxwv[:, :, 1, :].rearrange("p a (h m) -> p a h m", h=HPC)
            for p_ in range(2):
                ssl = slice((2 * pi + p_) * P, (2 * pi + p_ + 1) * P)
                for c in range(NCH):
                    nc.tensor.matmul(
                        kxwv[:, p_, 0, :], kt[:, c, ssl], wko[:, c, :],
                        start=(c == 0), stop=(c == NCH - 1),
                    )
                for c in range(NCH):
                    nc.tensor.matmul(
                        kxwv[:, p_, 1, :], vt[:, c, ssl], wv[:, c, :],
                        start=(c == 0), stop=(c == NCH - 1),
                    )
            if pi >= 2:
                emit_kv(pi - 2)

            # kp layout [p_, h, sign, m]: head h's 128 features contiguous
            # in [+64, -64] order so the kv matmul stationary slice is 2D.
            kp = work.tile([P, 2, HPC, 2, DK], dt.bfloat16, tag="kp", bufs=3)
            nc.scalar.activation(kp[:, :, :, 0, :], kxw4, AF.Exp, scale=1.0)

            # square/rho ordered before the second kp exp so the
            # rho -> v1 -> kv chain unblocks one ACT op earlier; the kv
            # matmuls consume kp two pairs later, so kp- can wait.
            sqsc = work.tile([P, 2, HPC, DK], dt.float32, tag="sqsc")
            nc.scalar.activation(sqsc[:], kxw4, AF.Square)
            ksqr = work.tile([P, 2, HPC, 1], dt.float32, tag="ksqr")
            nc.vector.reduce_sum(ksqr[:], sqsc[:], axis=mybir.AxisListType.X)
            rho = work.tile([P, 2, HPC, 1], dt.float32, tag="rho")
            nc.scalar.activation(rho[:], ksqr[:], AF.Exp, scale=-1.0 / 128.0)
            nc.scalar.activation(kp[:, :, :, 1, :], kxw4, AF.Exp, scale=-1.0)

            stage[pi] = {"v_ps": v_ps, "rho": rho, "kp": kp}
            if pi >= 1:
                emit_v1(pi - 1)

            if pi % 2 == 1:
                emit_q_chunk(pi // 2)

        emit_v1(NP_ - 1)
        emit_kv(NP_ - 2)
        emit_kv(NP_ - 1)
        nc.vector.tensor_copy(kv_sb[:], kv_big[:])
        # Build kvz via a DRAM round-trip (cross-partition moves need DMA;
        # matmul operands must then sit at partition base 0).
        nc.vector.memset(kvz[:], 0.0)
        nc.sync.dma_start(out=kvs_d.ap(), in_=kv_sb[:])
        for sg in range(2):
            for hf in range(2):
                dsl = slice(hf * (DV + 1), (hf + 1) * (DV + 1))
                nc.sync.dma_start(
                    out=kvz[hf * DK:(hf + 1) * DK, sg, :, dsl],
                    in_=kvs_d.ap().rearrange(
                        "p (pr hf) d -> p hf pr d", hf=2)[
                        sg * DK:(sg + 1) * DK, hf, :, :])

        # ---------------- tail: qkv + normalize + store ---------------------
        # Per-t-tile qkv through 5 rotating 1-bank PSUM tiles; normalize
        # alternates DVE / ACT; output DMAs spread across SP + ACT queues.
        out_r = out_d.ap().rearrange(
            "(h tc tt p d) -> h tc p tt d", h=HPC, tc=TC, tt=4, p=P, d=DV)
        qkv_tags = ["qx", "qx", "qx", "kxwv", "kxwv"]
        qkv_bufs = {"qx": 3, "kxwv": 2}
        qkv_i = 0
        for tcx in range(TC):
            qpt = qpt_tiles[tcx]
            oacc = work.tile([P, HPC, 4, DV], dt.float32, tag="oacc", bufs=4)
            oacc4 = oacc[:].rearrange("p (pr hf) tt d -> p pr hf tt d", hf=2)
            for tt in range(4):
                tg = qkv_tags[qkv_i % 5]
                qkv_i += 1
                # [pr, hd, 65]: one matmul per (pair, sign) yields both
                # heads of the pair via the block-diagonal kvz rhs.
                qkv = psum.tile([P, 2, 2, DV + 1], dt.float32, tag=tg,
                                bufs=qkv_bufs[tg], name="qkv")
                ttsl = slice(tt * P, (tt + 1) * P)
                for pr in range(2):
                    for sg in range(2):
                        nc.tensor.matmul(
                            qkv[:, pr, :, :],
                            qpt[:, pr, sg, ttsl],
                            kvz[:, sg, pr, :],
                            start=(sg == 0), stop=(sg == 1),
                        )
                recip = work.tile([P, 2, 2, 1], dt.float32, tag="recip")
                nc.vector.reciprocal(recip[:], qkv[:, :, :, DV:DV + 1])
                nc.vector.tensor_mul(
                    oacc4[:, :, :, tt, :], qkv[:, :, :, 0:DV],
                    recip[:].broadcast_to([P, 2, 2, DV]),
                )
            out_qs = [nc.sync, nc.scalar, nc.scalar, nc.sync]
            for h in range(HPC):
                out_qs[h].dma_start(out=out_r[h, tcx], in_=oacc[:, h, :, :])

def _get_program(reps=1):
    if reps not in _CACHE:
        _CACHE[reps] = _build_program(reps)
    return _CACHE[reps]


def _prep_core_inputs(query, value, key, wqo, wko, wv_w, core):
    b, hg = core // 2, core % 2
    hs = slice(hg * HPC, (hg + 1) * HPC)

    qT = np.ascontiguousarray(query[b].T.astype(np.float16))   # (512, 4096)
    kT = np.ascontiguousarray(key[b].T.astype(np.float16))
    vT = np.ascontiguousarray(value[b].T.astype(np.float16))

    wqp_c = np.concatenate(list(wqo[hs]), axis=1)              # (512, 256)
    wqp_c = np.ascontiguousarray(
        wqp_c.reshape(NCH, P, HPC * DK).astype(np.float16))
    wko_c = np.concatenate(list(wko[hs]), axis=1)              # (512, 256)
    wko_c = np.ascontiguousarray(
        wko_c.reshape(NCH, P, HPC * DK).astype(np.float16))
    wv_c = np.concatenate(list(wv_w[hs]), axis=1)              # (512, 256)
    wv_c = np.ascontiguousarray(
        wv_c.reshape(NCH, P, HPC * DV).astype(np.float16))

    return {"qt": qT, "kt": kT, "vt": vT,
            "wqp": wqp_c, "wko": wko_c, "wv": wv_c}


def kernel(query, value, key, wq, wv, wk, omega):
    from concourse.bass_utils import run_bass_kernel_spmd

    query = np.asarray(query, np.float32)
    value = np.asarray(value, np.float32)
    key = np.asarray(key, np.float32)
    wq = np.asarray(wq, np.float32)
    wv = np.asarray(wv, np.float32)
    wk = np.asarray(wk, np.float32)
    omega = np.asarray(omega, np.float32)

    nc = _get_program()

    wqo = np.einsum("hfk,mk->hfm", wq, omega)                  # (8, 512, 64)
    wko = np.einsum("hfk,mk->hfm", wk, omega)

    in_maps = [
        _prep_core_inputs(query, value, key, wqo, wko, wv, core)
        for core in range(8)
    ]
    res = run_bass_kernel_spmd(nc, in_maps, core_ids=list(range(8)))

    out = np.empty((B, T, D), np.float32)
    for core in range(8):
        b, hg = core // 2, core % 2
        out[b, hg * 2048:(hg + 1) * 2048, :] = (
            res.results[core]["out"].reshape(2048, 512))
    return out


# revision 50
# speedup vs baseline: 1030.7085x; 1.0159x over previous
"""Multi-head linear attention (Performer/FAVOR+) Bass kernel for 8x TRN2 cores.

Sharding: 8 cores = 4 batches x 2 head-groups. Core c handles batch c//2 and
heads [4*(c%2), 4*(c%2)+4).

Math notes (exact rewrites of the reference, not approximations):
  - omega is sqrt(64) * orthogonal, so Omega @ Omega.T = 64*I. Hence
    0.5*||q||^2 = ||q @ Omega.T||^2 / 128: the squared-sum term is computed
    from xw itself and the plain q/k projections are never needed.
  - The per-row scale exp(-sq_t) on phi(q), the global 1/sqrt(128) scale and
    (approximately) the +EPS term all cancel in out = qkv[..,:64]/qkv[..,64],
    so the q-side feature map is just exp(+-xw) (minus sign folded into the
    wqpm weights).
  - The k-side scale rho_s = exp(-ksq_s) is folded into v1 = [v, 1]*rho so
    kp is also just exp(+-kxw) and no per-column bias is needed.

Layouts: inputs are pre-transposed to f-major fp16 on the host (sharding /
layout prep). All projections contract f=512 over 4 chunks of 128 partitions.
kp/v1/qp/kv are bf16; PSUM accumulation is fp32.

Schedule: phase-Q projection work (qxw matmuls + exp) is interleaved into the
KV s-loop to fill dependency stalls; the qkv/normalize tail runs last.
PSUM budget (8 banks): big(4: kv accums, then qkv tiles) + kxw(1) + vps(1)
+ qx(2).
"""

import sys

import numpy as np

for _p in ("/opt/trn_rl_repo", "/root/.axon_site/_ro/trn_rl_repo"):
    try:
        import concourse  # noqa: F401
        break
    except ImportError:
        if _p not in sys.path:
            sys.path.insert(0, _p)

B, T, D, H = 4, 4096, 512, 8
DK = DV = 64
HPC = 4            # heads per core
NCH = 4            # f chunks (512 / 128)
P = 128
ST = T // P        # 32 s-tiles
TC = 8             # t chunks
TCW = T // TC      # 512

_CACHE = {}


def _build_program(reps=1):
    import concourse.mybir as mybir
    import concourse.tile as tile
    from concourse import bacc
    from contextlib import ExitStack

    dt = mybir.dt
    AF = mybir.ActivationFunctionType

    nc = bacc.Bacc("TRN2", target_bir_lowering=False, debug=False)

    qt_d = nc.dram_tensor("qt", [D, T], dt.float16, kind="ExternalInput")
    kt_d = nc.dram_tensor("kt", [D, T], dt.float16, kind="ExternalInput")
    vt_d = nc.dram_tensor("vt", [D, T], dt.float16, kind="ExternalInput")
    wqp_d = nc.dram_tensor("wqp", [NCH, P, HPC * DK], dt.float16, kind="ExternalInput")
    wko_d = nc.dram_tensor("wko", [NCH, P, HPC * DK], dt.float16, kind="ExternalInput")
    wv_d = nc.dram_tensor("wv", [NCH, P, HPC * DV], dt.float16, kind="ExternalInput")
    out_d = nc.dram_tensor("out", [HPC * T * DV], dt.float32, kind="ExternalOutput")
    kvs_d = nc.dram_tensor("kvs", [P, HPC, DV + 1], dt.bfloat16, kind="Internal")

    with tile.TileContext(nc) as tc, ExitStack() as ctx:
        const = ctx.enter_context(tc.tile_pool(name="const", bufs=1))
        work = ctx.enter_context(tc.tile_pool(name="work", bufs=3))
        psum = ctx.enter_context(tc.tile_pool(name="psum", bufs=1, space="PSUM"))
        for _rep in range(reps):
            _emit_body(nc, tc, const, work, psum, mybir, dt, AF,
                       qt_d, kt_d, vt_d, wqp_d, wko_d, wv_d, out_d, kvs_d)

    nc.compile()
    return nc


def _emit_body(nc, tc, const, work, psum, mybir, dt, AF,
               qt_d, kt_d, vt_d, wqp_d, wko_d, wv_d, out_d, kvs_d):
    if True:

        # persistent SBUF residents
        qt = const.tile([P, NCH, T], dt.float16)
        kt = const.tile([P, NCH, T], dt.float16)
        vt = const.tile([P, NCH, T], dt.float16)
        wqp = const.tile([P, NCH, HPC * DK], dt.float16)
        wko = const.tile([P, NCH, HPC * DK], dt.float16)
        wv = const.tile([P, NCH, HPC * DV], dt.float16)
        kv_sb = const.tile([P, HPC, DV + 1], dt.bfloat16)
        # Block-diagonal kv for the packed qkv matmuls: for pair pr and
        # sign s, a [128, 130] rhs whose top-left 64x65 block is kv of head
        # 2pr and bottom-right 64x65 block is kv of head 2pr+1 (zeros
        # elsewhere). One matmul against the packed qpt tile then yields
        # both heads' qkv side by side, with all operands at partition 0.
        kvz = const.tile([P, 2, 2, 2 * (DV + 1)], dt.bfloat16)

        # Load order matters: k/v weights and the first k/v columns unblock
        # the first loop iterations; wqpm/qt follow for the first q-chunk;
        # later 512-column segments stream in interleaved in the order the
        # loop consumes them.
        qt_r = qt_d.ap().rearrange("(c p) t -> p c t", p=P)
        kt_r = kt_d.ap().rearrange("(c p) t -> p c t", p=P)
        vt_r = vt_d.ap().rearrange("(c p) t -> p c t", p=P)

        nc.sync.dma_start(out=wko[:], in_=wko_d.ap().rearrange("c p n -> p c n"))
        nc.sync.dma_start(out=wv[:], in_=wv_d.ap().rearrange("c p n -> p c n"))
        nc.sync.dma_start(out=kt[:, :, 0:256], in_=kt_r[:, :, 0:256])
        nc.sync.dma_start(out=vt[:, :, 0:256], in_=vt_r[:, :, 0:256])
        nc.sync.dma_start(out=wqp[:], in_=wqp_d.ap().rearrange("c p n -> p c n"))
        nc.sync.dma_start(out=qt[:, :, 0:512], in_=qt_r[:, :, 0:512])
        nc.sync.dma_start(out=kt[:, :, 256:512], in_=kt_r[:, :, 256:512])
        nc.sync.dma_start(out=vt[:, :, 256:512], in_=vt_r[:, :, 256:512])
        for psl in (slice(512, 1536), slice(1536, 2560), slice(2560, 3584),
                    slice(3584, T)):
            nc.sync.dma_start(out=kt[:, :, psl], in_=kt_r[:, :, psl])
            nc.sync.dma_start(out=vt[:, :, psl], in_=vt_r[:, :, psl])
            nc.sync.dma_start(out=qt[:, :, psl], in_=qt_r[:, :, psl])

        # Single-bank PSUM accumulator: kv[h] at columns [h*65, h*65+65).
        # first_mm clears has_written at BANK granularity, so interleaved
        # per-head groups must NOT use start=True: memset the bank once and
        # accumulate from the first matmul (overwrite-on-clear / add-on-set
        # per-element semantics make both stale-flag cases correct).
        kv_big = psum.tile([P, HPC, DV + 1], dt.float32, tag="kvacc", bufs=1)
        nc.vector.memset(kv_big[:], 0.0)

        qpt_tiles = []

        def emit_q_pair(tcx, pr):
            # One head-pair of q-projection per loop pair (instead of a
            # whole chunk every other pair): spreads ACT work evenly so the
            # next pair's kp/rho chain isn't stuck behind a 4-exp burst.
            tsl = slice(tcx * TCW, (tcx + 1) * TCW)
            if pr == 0:
                # [pair, sign, t]: partitions hold (head 2p: m 0..63,
                # head 2p+1: m 0..63); sign 0 = exp(+xw), 1 = exp(-xw).
                qpt = work.tile([P, 2, 2, TCW], dt.bfloat16, tag="qpt",
                                bufs=TC, name=f"qpt{tcx}")
                qpt_tiles.append(qpt)
            qpt = qpt_tiles[tcx]
            qx = psum.tile([P, TCW], dt.float32, tag="qx", bufs=3,
                           name="qx")
            for c in range(NCH):
                nc.tensor.matmul(
                    qx[:], wqp[:, c, pr * P:(pr + 1) * P], qt[:, c, tsl],
                    start=(c == 0), stop=(c == NCH - 1),
                )
            nc.scalar.activation(qpt[:, pr, 0, :], qx[:], AF.Exp)
            nc.scalar.activation(qpt[:, pr, 1, :], qx[:], AF.Exp,
                                 scale=-1.0)

        # ---------------- phase KV (with q-projection work interleaved) -----
        # Two s-tiles are processed per "pair" iteration (halves the DVE/ACT
        # op count). Engines execute their queues IN ORDER, so cross-engine
        # dependencies are software-pipelined: v1 (needs rho from ACT) is
        # emitted one pair late on DVE, the kv matmuls (need v1) one pair
        # later still on PE. kp-exps are emitted before rho on ACT since
        # they only need kxw.
        NP_ = ST // 2    # 16 pairs
        stage = {}       # pi -> dict of tiles

        def emit_v1(pi):
            st_ = stage[pi]
            v1 = work.tile([P, 2, HPC, DV + 1], dt.bfloat16, tag="v1",
                           name="v1")
            nc.vector.tensor_mul(
                v1[:, :, :, 0:DV], st_["v_ps"],
                st_["rho"][:].broadcast_to([P, 2, HPC, DV])
            )
            nc.vector.tensor_copy(v1[:, :, :, DV:DV + 1], st_["rho"][:])
            st_["v1"] = v1

        def emit_kv(pi):
            st_ = stage.pop(pi)
            for p_ in range(2):
                si = 2 * pi + p_
                for h in range(HPC):
                    # lhsT = [s, 128]: head h's features in [+64, -64] order
                    # (contiguous, matches kv row order).
                    nc.tensor.matmul(
                        kv_big[:, h, :], st_["kp"][:, p_, h, :, :],
                        st_["v1"][:, p_, h, :],
                        start=False, stop=(si == ST - 1),
                        skip_group_check=True,
                    )

        for pi in range(NP_):
            # kxw and v as contiguous 256-wide slabs per s-tile:
            # [:, p_, 0, :] = kxw (4h x 64), [:, p_, 1, :] = v (4h x 64).
            kxwv = psum.tile([P, 2, 2, HPC * DK], dt.float32, tag="kxwv",
                             bufs=2, name="kxwv")
            kxw = kxwv[:, :, 0, :]
            kxw4 = kxw.rearrange("p a (h m) -> p a h m", h=HPC)
            v_ps = kxwv[:, :, 1, :].rearrange("p a (h m) -> p a h m", h=HPC)
            for p_ in range(2):
                ssl = slice((2 * pi + p_) * P, (2 * pi + p_ + 1) * P)
                for c in range(NCH):
                    nc.tensor.matmul(
                        kxwv[:, p_, 0, :], kt[:, c, ssl], wko[:, c, :],
                        start=(c == 0), stop=(c == NCH - 1),
                    )
                for c in range(NCH):
                    nc.tensor.matmul(
                        kxwv[:, p_, 1, :], vt[:, c, ssl], wv[:, c, :],
                        start=(c == 0), stop=(c == NCH - 1),
                    )
            if pi >= 2:
                emit_kv(pi - 2)

            # kp layout [p_, h, sign, m]: head h's 128 features contiguous
            # in [+64, -64] order so the kv matmul stationary slice is 2D.
            kp = work.tile([P, 2, HPC, 2, DK], dt.bfloat16, tag="kp", bufs=3)
            nc.scalar.activation(kp[:, :, :, 0, :], kxw4, AF.Exp, scale=1.0)

            # square/rho ordered before the second kp exp so the
            # rho -> v1 -> kv chain unblocks one ACT op earlier; the kv
            # matmuls consume kp two pairs later, so kp- can wait.
            sqsc = work.tile([P, 2, HPC, DK], dt.float32, tag="sqsc")
            nc.scalar.activation(sqsc[:], kxw4, AF.Square)
            ksqr = work.tile([P, 2, HPC, 1], dt.float32, tag="ksqr")
            nc.vector.reduce_sum(ksqr[:], sqsc[:], axis=mybir.AxisListType.X)
            rho = work.tile([P, 2, HPC, 1], dt.float32, tag="rho")
            nc.scalar.activation(rho[:], ksqr[:], AF.Exp, scale=-1.0 / 128.0)
            nc.scalar.activation(kp[:, :, :, 1, :], kxw4, AF.Exp, scale=-1.0)

            stage[pi] = {"v_ps": v_ps, "rho": rho, "kp": kp}
            if pi >= 1:
                emit_v1(pi - 1)

            emit_q_pair(pi // 2, pi % 2)

        emit_v1(NP_ - 1)
        emit_kv(NP_ - 2)
        emit_kv(NP_ - 1)
        nc.vector.tensor_copy(kv_sb[:], kv_big[:])
        # Build kvz via a DRAM round-trip (cross-partition moves need DMA;
        # matmul operands must then sit at partition base 0).
        nc.vector.memset(kvz[:], 0.0)
        nc.sync.dma_start(out=kvs_d.ap(), in_=kv_sb[:])
        for sg in range(2):
            for hf in range(2):
                dsl = slice(hf * (DV + 1), (hf + 1) * (DV + 1))
                nc.sync.dma_start(
                    out=kvz[hf * DK:(hf + 1) * DK, sg, :, dsl],
                    in_=kvs_d.ap().rearrange(
                        "p (pr hf) d -> p hf pr d", hf=2)[
                        sg * DK:(sg + 1) * DK, hf, :, :])

        # ---------------- tail: qkv + normalize + store ---------------------
        # Per-t-tile qkv through 5 rotating 1-bank PSUM tiles; normalize
        # alternates DVE / ACT; output DMAs spread across SP + ACT queues.
        out_r = out_d.ap().rearrange(
            "(h tc tt p d) -> h tc p tt d", h=HPC, tc=TC, tt=4, p=P, d=DV)
        qkv_tags = ["qx", "qx", "qx", "kxwv", "kxwv"]
        qkv_bufs = {"qx": 3, "kxwv": 2}
        qkv_i = 0
        for tcx in range(TC):
            qpt = qpt_tiles[tcx]
            oacc = work.tile([P, HPC, 4, DV], dt.float32, tag="oacc", bufs=4)
            oacc4 = oacc[:].rearrange("p (pr hf) tt d -> p pr hf tt d", hf=2)
            for tt in range(4):
                tg = qkv_tags[qkv_i % 5]
                qkv_i += 1
                # [pr, hd, 65]: one matmul per (pair, sign) yields both
                # heads of the pair via the block-diagonal kvz rhs.
                qkv = psum.tile([P, 2, 2, DV + 1], dt.float32, tag=tg,
                                bufs=qkv_bufs[tg], name="qkv")
                ttsl = slice(tt * P, (tt + 1) * P)
                for pr in range(2):
                    for sg in range(2):
                        nc.tensor.matmul(
                            qkv[:, pr, :, :],
                            qpt[:, pr, sg, ttsl],
                            kvz[:, sg, pr, :],
                            start=(sg == 0), stop=(sg == 1),
                        )
                recip = work.tile([P, 2, 2, 1], dt.float32, tag="recip")
                nc.vector.reciprocal(recip[:], qkv[:, :, :, DV:DV + 1])
                nc.vector.tensor_mul(
                    oacc4[:, :, :, tt, :], qkv[:, :, :, 0:DV],
                    recip[:].broadcast_to([P, 2, 2, DV]),
                )
            out_qs = [nc.sync, nc.scalar, nc.scalar, nc.sync]
            for h in range(HPC):
                out_qs[h].dma_start(out=out_r[h, tcx], in_=oacc[:, h, :, :])

def _get_program(reps=1):
    if reps not in _CACHE:
        _CACHE[reps] = _build_program(reps)
    return _CACHE[reps]


def _prep_core_inputs(query, value, key, wqo, wko, wv_w, core):
    b, hg = core // 2, core % 2
    hs = slice(hg * HPC, (hg + 1) * HPC)

    qT = np.ascontiguousarray(query[b].T.astype(np.float16))   # (512, 4096)
    kT = np.ascontiguousarray(key[b].T.astype(np.float16))
    vT = np.ascontiguousarray(value[b].T.astype(np.float16))

    wqp_c = np.concatenate(list(wqo[hs]), axis=1)              # (512, 256)
    wqp_c = np.ascontiguousarray(
        wqp_c.reshape(NCH, P, HPC * DK).astype(np.float16))
    wko_c = np.concatenate(list(wko[hs]), axis=1)              # (512, 256)
    wko_c = np.ascontiguousarray(
        wko_c.reshape(NCH, P, HPC * DK).astype(np.float16))
    wv_c = np.concatenate(list(wv_w[hs]), axis=1)              # (512, 256)
    wv_c = np.ascontiguousarray(
        wv_c.reshape(NCH, P, HPC * DV).astype(np.float16))

    return {"qt": qT, "kt": kT, "vt": vT,
            "wqp": wqp_c, "wko": wko_c, "wv": wv_c}


def kernel(query, value, key, wq, wv, wk, omega):
    from concourse.bass_utils import run_bass_kernel_spmd

    query = np.asarray(query, np.float32)
    value = np.asarray(value, np.float32)
    key = np.asarray(key, np.float32)
    wq = np.asarray(wq, np.float32)
    wv = np.asarray(wv, np.float32)
    wk = np.asarray(wk, np.float32)
    omega = np.asarray(omega, np.float32)

    nc = _get_program()

    wqo = np.einsum("hfk,mk->hfm", wq, omega)                  # (8, 512, 64)
    wko = np.einsum("hfk,mk->hfm", wk, omega)

    in_maps = [
        _prep_core_inputs(query, value, key, wqo, wko, wv, core)
        for core in range(8)
    ]
    res = run_bass_kernel_spmd(nc, in_maps, core_ids=list(range(8)))

    out = np.empty((B, T, D), np.float32)
    for core in range(8):
        b, hg = core // 2, core % 2
        out[b, hg * 2048:(hg + 1) * 2048, :] = (
            res.results[core]["out"].reshape(2048, 512))
    return out
